# revision 27
# baseline (speedup 1.0000x reference)
"""MHA kernel for 8 trn2 NeuronCores (self-contained).

Reference computation (fp32):
    qh = split_heads(q @ Wq + bq); kh, vh likewise
    w  = softmax(qh @ kh^T / 8 + mask * -1e9)     # [B,H,Sq,Sk]
    out = merge_heads(w @ vh) @ Wo + bo           # [B,S,D]
    returns (out, w)

Sharding: data-parallel over batch (2) x tensor-parallel over heads
(4 heads/core) = 8 cores. Each core computes, for its batch b and its
4 heads:
    Q^T, K^T  [256, seq]  (head-dim on partitions)  via Wq/Wk col-shards
    V         [seq, 4*65] (ones column per head appended -> softmax denom)
    E^T = exp(scores^T)   in [k, q] orientation (no max-subtraction;
                          scores are O(5) here so exp is safe in fp32)
    AV: psum[65, q] = V~_h^T @ E^T_h  -> rows 0-63 = O^T_h, row 64 = denom
    wT output  [4, seq, seq] = UNNORMALIZED E^T   (host: transpose + x recip)
    recip output [4, seq]    = 1/denominators
    outT output [D, seq] = Wo_s^T @ (O^T * recip) (host transposes+reduces)

All matmuls run as float32r (fp32 data, PE fast mode, ~1e-4 rounding).
E^T is written to DRAM straight after exp (before normalization), grouped
4 k-chunks per DMA (2 MiB) so the writes saturate HBM; the host applies
the 1/denom scaling during its transpose pass.
"""

import numpy as np

import concourse.bacc as bacc
import concourse.mybir as mybir
import concourse.tile as tile
from concourse.bass_utils import run_bass_kernel_spmd

F32 = mybir.dt.float32
F32R = mybir.dt.float32r
AF = mybir.ActivationFunctionType
MUL = mybir.AluOpType.mult

# Problem constants (hardcoded per contract)
BS = 2
SEQ = 2048
D = 1024
H = 16
HD = 64  # head dim
N_CORES = 8
TP = 4  # tensor-parallel cores per batch
HPC = H // TP  # heads per core = 4
DHC = HPC * HD  # 256 head-dims per core
KT = D // 128  # 8 dmodel k-tiles
SCALE = 1.0 / float(np.sqrt(np.float32(HD)))  # 0.125

GRP = 2  # k-chunks per E^T group (one 1 MiB w-write DMA)


def build_kernel(seq=SEQ, big_bufs=12, early=5):
    nkc = seq // 128          # 128-wide k chunks
    ng = nkc // GRP           # e^T groups per (head, q-half)
    qh = 1024 if seq >= 1024 else seq   # q-half (attention block width)
    nj = seq // qh
    n5h = qh // 512           # 512-chunks per q-half
    n5 = seq // 512

    nc = bacc.Bacc(None, target_bir_lowering=False)

    # ---- DRAM I/O (per-core shard) ----
    xqT = nc.dram_tensor("xqT", [D, seq], F32R, kind="ExternalInput")
    xkT = nc.dram_tensor("xkT", [D, seq], F32R, kind="ExternalInput")
    xvT = nc.dram_tensor("xvT", [D, seq], F32R, kind="ExternalInput")
    wq = nc.dram_tensor("wq", [D, DHC], F32R, kind="ExternalInput")
    wk = nc.dram_tensor("wk", [D, DHC], F32R, kind="ExternalInput")
    wv = nc.dram_tensor("wv", [D, DHC], F32R, kind="ExternalInput")
    wo = nc.dram_tensor("wo", [DHC, D], F32R, kind="ExternalInput")
    bq = nc.dram_tensor("bq", [DHC], F32, kind="ExternalInput")
    bk = nc.dram_tensor("bk", [DHC], F32, kind="ExternalInput")
    bv = nc.dram_tensor("bv", [DHC], F32R, kind="ExternalInput")
    maskneg = nc.dram_tensor("maskneg", [seq], F32, kind="ExternalInput")

    wT_out = nc.dram_tensor("wT", [HPC, seq, seq], F32, kind="ExternalOutput")
    recip_out = nc.dram_tensor("recip", [HPC, seq], F32, kind="ExternalOutput")
    outT = nc.dram_tensor("outT", [D, seq], F32, kind="ExternalOutput")

    with tile.TileContext(nc) as tc:
        with (
            tc.tile_pool(name="consts", bufs=1) as consts,
            tc.tile_pool(name="qkv", bufs=1) as qkv_pool,
            tc.tile_pool(name="small", bufs=1) as small,
            tc.tile_pool(name="big", bufs=big_bufs) as big_pool,
            tc.tile_pool(name="ps", bufs=2, space="PSUM") as ps,
        ):

            # persistent activation tensors
            qT_sb = qkv_pool.tile([128, 2, seq], F32R, tag="qT")
            kT_sb = qkv_pool.tile([128, 2, seq], F32R, tag="kT")
            v_sb = qkv_pool.tile([128, nkc, HPC * 65], F32R, tag="v")

            et_store = {}
            et_pool = big_pool

            deferred_dmas = []

            def emit_group(j, h, g, defer=False):
                pq = 64 * (h % 2)
                qt_h = qT_sb[pq:pq + 64, h // 2, :]
                kt_h = kT_sb[pq:pq + 64, h // 2, :]
                eg = et_pool.tile([128, GRP, qh], F32R, tag="big",
                                  name=f"eg{j}_{h}_{g}")
                for kg in range(GRP):
                    kc = g * GRP + kg
                    pscore = ps.tile([128, qh], F32, tag="ps_big",
                                     name="pscore")
                    for v5 in range(n5h):
                        nc.tensor.matmul(
                            pscore[:, v5 * 512:(v5 + 1) * 512],
                            kt_h[:, kc * 128:(kc + 1) * 128],
                            qt_h[:, j * qh + v5 * 512:j * qh + (v5 + 1) * 512],
                            start=True,
                            stop=True,
                        )
                    nc.scalar.activation(
                        eg[:, kg, :],
                        pscore[:],
                        AF.Exp,
                        bias=mask_sb[:, kc:kc + 1],
                        scale=SCALE,
                    )
                # unnormalized w^T group -> DRAM
                dst = wT_out[h, g * GRP * 128:(g + 1) * GRP * 128,
                             j * qh:(j + 1) * qh] \
                    .rearrange("(c p) q -> p c q", p=128)
                if defer:
                    deferred_dmas.append((dst, eg))
                else:
                    nc.sync.dma_start(dst, eg[:].bitcast(F32))
                et_store[(j, h, g)] = eg

            # ---- phase A: projections (Q^T, K^T first, V last) ----
            xt_pool = big_pool
            with tc.tile_pool(name="projw", bufs=1) as projw:
                wq_sb = projw.tile([128, KT, DHC], F32R, tag="wq")
                nc.sync.dma_start(wq_sb[:], wq.rearrange("(k p) n -> p k n", p=128))
                bq_sb = consts.tile([128, 2], F32, tag="bq")
                nc.sync.dma_start(bq_sb[:], bq.rearrange("(k p) -> p k", p=128))
                bk_sb = consts.tile([128, 2], F32, tag="bk")
                nc.sync.dma_start(bk_sb[:], bk.rearrange("(k p) -> p k", p=128))
                bv_sb = consts.tile([1, DHC], F32R, tag="bv")
                nc.sync.dma_start(bv_sb[:], bv[None, :])
                mask_sb = consts.tile([128, nkc], F32, tag="mask")
                nc.sync.dma_start(mask_sb[:],
                                  maskneg.rearrange("(c p) -> p c", p=128))
                onesf_sb = consts.tile([1, 128], F32, tag="onesf")
                nc.vector.memset(onesf_sb[:], 1.0)
                ones_sb = consts.tile([1, 128], F32R, tag="ones")
                nc.vector.tensor_copy(ones_sb[:], onesf_sb[:])
                onevf_sb = consts.tile([128, HPC], F32, tag="onevf")
                nc.vector.memset(onevf_sb[:], 1.0)
                onesv_sb = consts.tile([128, HPC], F32R, tag="onesv")
                nc.vector.tensor_copy(onesv_sb[:], onevf_sb[:])
                wk_sb = projw.tile([128, KT, DHC], F32R, tag="wk")
                nc.sync.dma_start(wk_sb[:], wk.rearrange("(k p) n -> p k n", p=128))
                wv_sb = projw.tile([128, KT, DHC], F32R, tag="wv")
                nc.sync.dma_start(wv_sb[:],
                                  wv.rearrange("(k p) n -> p k n", p=128))

                def proj_qk(xT_d, w_sb, b_sb, dst, name, ms):
                    xt = [xt_pool.tile([128, seq], F32R, tag="big",
                                       name=f"xt{name}{k}")
                          for k in range(KT)]
                    for k in range(KT):
                        nc.sync.dma_start(xt[k][:],
                                          xT_d[k * 128:(k + 1) * 128, :])
                    for m in ms:
                        for nn in range(nj):
                            pt = ps.tile([128, qh], F32, tag="ps_big",
                                         name="pt")
                            for v5 in range(n5h):
                                for k in range(KT):
                                    nc.tensor.matmul(
                                        pt[:, v5 * 512:(v5 + 1) * 512],
                                        w_sb[:, k, m * 128:(m + 1) * 128],
                                        xt[k][:, nn * qh + v5 * 512:
                                              nn * qh + (v5 + 1) * 512],
                                        start=(k == 0),
                                        stop=(k == KT - 1),
                                    )
                            nc.vector.tensor_scalar_add(
                                dst[:, m, nn * qh:(nn + 1) * qh],
                                pt[:],
                                b_sb[:, m:m + 1],
                            )
                        yield m

                # Q^T then K^T fully
                for _ in proj_qk(xqT, wq_sb, bq_sb, qT_sb, "q", (0, 1)):
                    pass
                for _ in proj_qk(xkT, wk_sb, bk_sb, kT_sb, "k", (0, 1)):
                    pass

                # V: [seq, HPC*65] with ones column per head; the first
                # attention block's score groups (no V dependency) are
                # emitted alongside so their w-writes overlap the xv DMA
                xt = [xt_pool.tile([128, seq], F32R, tag="big",
                                   name=f"xtv{k}")
                      for k in range(KT)]
                for k in range(KT):
                    nc.sync.dma_start(xt[k][:], xvT[k * 128:(k + 1) * 128, :])
                for g0 in range(min(early, ng)):
                    emit_group(0, 0, g0)
                for ms in range(nkc):
                    pv = ps.tile([128, DHC], F32, tag="ps_av")
                    for k in range(KT):
                        nc.tensor.matmul(
                            pv[:],
                            xt[k][:, ms * 128:(ms + 1) * 128],
                            wv_sb[:, k, :],
                            start=(k == 0),
                            stop=False,
                        )
                    nc.tensor.matmul(
                        pv[:], ones_sb[:], bv_sb[:], start=False, stop=True
                    )
                    nc.vector.tensor_copy(
                        v_sb[:, ms, :]
                        .rearrange("p (h d) -> p h d", h=HPC)[:, :, :HD],
                        pv[:].rearrange("p (h d) -> p h d", h=HPC),
                    )
                    nc.vector.tensor_copy(
                        v_sb[:, ms, :]
                        .rearrange("p (h d) -> p h d", h=HPC)[:, :, HD:],
                        onesv_sb[:, :, None],
                    )



            # ---- phase B: attention ----
            with tc.tile_pool(name="outp", bufs=2) as out_pool:
                wo_sb = out_pool.tile([128, 2, D], F32R, tag="wo", bufs=1)
                nc.sync.dma_start(wo_sb[:],
                                  wo.rearrange("(k p) n -> p k n", p=128))
                a_sb = out_pool.tile([128, 2, seq], F32R, tag="aT", bufs=1)
                for j in range(nj):
                    for h in range(HPC):
                        pq = 64 * (h % 2)
                        qt_h = qT_sb[pq:pq + 64, h // 2, :]
                        kt_h = kT_sb[pq:pq + 64, h // 2, :]
                        last = (j == nj - 1 and h == HPC - 1)
                        for g in range(ng):
                            if (j, h, g) not in et_store:
                                emit_group(j, h, g, defer=last and g >= ng - 2)
                        ets = [et_store.pop((j, h, g)) for g in range(ng)]

                        # AV + denominators
                        pav = ps.tile([65, qh], F32, tag="ps_av")
                        for g in range(ng):
                            for kg in range(GRP):
                                kc = g * GRP + kg
                                for v5 in range(n5h):
                                    nc.tensor.matmul(
                                        pav[:, v5 * 512:(v5 + 1) * 512],
                                        v_sb[:, kc, h * 65:(h + 1) * 65],
                                        ets[g][:, kg,
                                               v5 * 512:(v5 + 1) * 512],
                                        start=(kc == 0),
                                        stop=(kc == nkc - 1),
                                    )

                        # recip of denominators -> host + broadcast for A^T
                        recip = small.tile([1, qh], F32R, tag="recip")
                        with nc.allow_low_precision(
                            reason="f32r recip feeds f32r matmul broadcast"
                        ):
                            nc.vector.reciprocal(recip[:], pav[64:65, :])
                        nc.sync.dma_start(
                            recip_out[h, j * qh:(j + 1) * qh][None, :],
                            recip[:].bitcast(F32),
                        )
                        pb = ps.tile([64, qh], F32, tag="ps_av")
                        for v5 in range(n5h):
                            nc.tensor.matmul(
                                pb[:, v5 * 512:(v5 + 1) * 512],
                                ones_sb[:, :64],
                                recip[:, v5 * 512:(v5 + 1) * 512],
                                start=True,
                                stop=True,
                            )
                        rb = small.tile([64, qh], F32, tag="rb")
                        nc.vector.tensor_copy(rb[:], pb[:])
                        # normalize O^T into A^T (f32r, for out-projection)
                        nc.vector.tensor_tensor(
                            a_sb[pq:pq + 64, h // 2, j * qh:(j + 1) * qh],
                            pav[0:64, :],
                            rb[:],
                            MUL,
                        )

                    # ---- output projection for this q-half ----
                    for m in range(D // 128):
                        ot = out_pool.tile([128, qh], F32, tag="ot")
                        for v5 in range(n5h):
                            po = ps.tile([128, 512], F32, tag="ps_av")
                            for k in range(2):
                                nc.tensor.matmul(
                                    po[:],
                                    wo_sb[:, k, m * 128:(m + 1) * 128],
                                    a_sb[:, k, j * qh + v5 * 512:
                                         j * qh + (v5 + 1) * 512],
                                    start=(k == 0),
                                    stop=(k == 1),
                                )
                            nc.scalar.copy(ot[:, v5 * 512:(v5 + 1) * 512],
                                           po[:])
                        nc.sync.dma_start(
                            outT[m * 128:(m + 1) * 128, j * qh:(j + 1) * qh],
                            ot[:],
                        )
                        if deferred_dmas and m == 1:
                            for dst, eg in deferred_dmas:
                                nc.sync.dma_start(dst, eg[:].bitcast(F32))
                            deferred_dmas.clear()

    nc.compile()
    return nc


_NC_CACHE = {}


def _get_nc():
    if "nc" not in _NC_CACHE:
        _NC_CACHE["nc"] = build_kernel()
    return _NC_CACHE["nc"]


def shard_inputs(q, k, v, mask, Wq, bq, Wk, bk, Wv, bv, Wo):
    xT = {}
    mneg = {}
    for b in range(BS):
        xT[b] = (
            np.ascontiguousarray(q[b].T),
            np.ascontiguousarray(k[b].T),
            np.ascontiguousarray(v[b].T),
        )
        mneg[b] = np.ascontiguousarray(mask[b, 0, 0, :] * np.float32(-1e9))

    in_maps = []
    for c in range(N_CORES):
        b, t = divmod(c, TP)
        sl = slice(t * DHC, (t + 1) * DHC)
        in_maps.append(
            {
                "xqT": xT[b][0],
                "xkT": xT[b][1],
                "xvT": xT[b][2],
                "wq": np.ascontiguousarray(Wq[:, sl]),
                "wk": np.ascontiguousarray(Wk[:, sl]),
                "wv": np.ascontiguousarray(Wv[:, sl]),
                "wo": np.ascontiguousarray(Wo[sl, :]),
                "bq": np.ascontiguousarray(bq[sl]),
                "bk": np.ascontiguousarray(bk[sl]),
                "bv": np.ascontiguousarray(bv[sl]),
                "maskneg": mneg[b],
            }
        )
    return in_maps


def unshard_outputs(results, bo):
    out = np.empty((BS, SEQ, D), dtype=np.float32)
    w = np.empty((BS, H, SEQ, SEQ), dtype=np.float32)
    acc = np.zeros((BS, D, SEQ), dtype=np.float32)
    for c in range(N_CORES):
        b, t = divmod(c, TP)
        r = results[c]
        acc[b] += r["outT"]
        wt = r["wT"]  # [HPC, k, q] unnormalized
        rc = r["recip"]  # [HPC, q]
        for h in range(HPC):
            # w[q, k] = E^T[k, q].T * recip[q, None]
            np.multiply(wt[h].T, rc[h][:, None], out=w[b, t * HPC + h])
    for b in range(BS):
        out[b] = acc[b].T + bo[None, :]
    return out, w


def kernel(q, k, v, mask, Wq, bq, Wk, bk, Wv, bv, Wo, bo):
    args = [np.asarray(a, dtype=np.float32)
            for a in (q, k, v, mask, Wq, bq, Wk, bk, Wv, bv, Wo, bo)]
    q, k, v, mask, Wq, bq, Wk, bk, Wv, bv, Wo, bo = args

    nc = _get_nc()
    in_maps = shard_inputs(q, k, v, mask, Wq, bq, Wk, bk, Wv, bv, Wo)
    res = run_bass_kernel_spmd(nc, in_maps, list(range(N_CORES)))
    return unshard_outputs(res.results, bo)


# revision 33
# speedup vs baseline: 1.0271x; 1.0271x over previous
"""MHA kernel for 8 trn2 NeuronCores (self-contained).

Reference computation (fp32):
    qh = split_heads(q @ Wq + bq); kh, vh likewise
    w  = softmax(qh @ kh^T / 8 + mask * -1e9)     # [B,H,Sq,Sk]
    out = merge_heads(w @ vh) @ Wo + bo           # [B,S,D]
    returns (out, w)

Sharding: data-parallel over batch (2) x tensor-parallel over heads
(4 heads/core) = 8 cores. Each core computes, for its batch b and its
4 heads:
    Q^T, K^T  [256, seq]  (head-dim on partitions)  via Wq/Wk col-shards
    V         [seq, 4*65] (ones column per head appended -> softmax denom)
    E^T = exp(scores^T)   in [k, q] orientation (no max-subtraction;
                          scores are O(5) here so exp is safe in fp32)
    AV: psum[65, q] = V~_h^T @ E^T_h  -> rows 0-63 = O^T_h, row 64 = denom
    wT output  [4, seq, seq] = UNNORMALIZED E^T   (host: transpose + x recip)
    recip output [4, seq]    = 1/denominators
    outT output [D, seq] = Wo_s^T @ (O^T * recip) (host transposes+reduces)

All matmuls run as float32r (fp32 data, PE fast mode, ~1e-4 rounding).
E^T is written to DRAM straight after exp (before normalization), grouped
4 k-chunks per DMA (2 MiB) so the writes saturate HBM; the host applies
the 1/denom scaling during its transpose pass.
"""

import numpy as np

import concourse.bacc as bacc
import concourse.mybir as mybir
import concourse.tile as tile
from concourse.bass_utils import run_bass_kernel_spmd

F32 = mybir.dt.float32
F32R = mybir.dt.float32r
AF = mybir.ActivationFunctionType
MUL = mybir.AluOpType.mult

# Problem constants (hardcoded per contract)
BS = 2
SEQ = 2048
D = 1024
H = 16
HD = 64  # head dim
N_CORES = 8
TP = 4  # tensor-parallel cores per batch
HPC = H // TP  # heads per core = 4
DHC = HPC * HD  # 256 head-dims per core
KT = D // 128  # 8 dmodel k-tiles
SCALE = 1.0 / float(np.sqrt(np.float32(HD)))  # 0.125

GRP = 2  # k-chunks per E^T group (one 1 MiB w-write DMA)


def build_kernel(seq=SEQ, big_bufs=14, early=6, ndefer=6, pre_j=2):
    nkc = seq // 128          # 128-wide k chunks
    ng = nkc // GRP           # e^T groups per (head, q-half)
    qh = 1024 if seq >= 1024 else seq   # q-half (attention block width)
    nj = seq // qh
    n5h = qh // 512           # 512-chunks per q-half
    n5 = seq // 512

    nc = bacc.Bacc(None, target_bir_lowering=False)

    # ---- DRAM I/O (per-core shard) ----
    xqT = nc.dram_tensor("xqT", [D, seq], F32R, kind="ExternalInput")
    xkT = nc.dram_tensor("xkT", [D, seq], F32R, kind="ExternalInput")
    xvT = nc.dram_tensor("xvT", [D, seq], F32R, kind="ExternalInput")
    wq = nc.dram_tensor("wq", [D, DHC], F32R, kind="ExternalInput")
    wk = nc.dram_tensor("wk", [D, DHC], F32R, kind="ExternalInput")
    wv = nc.dram_tensor("wv", [D, DHC], F32R, kind="ExternalInput")
    wo = nc.dram_tensor("wo", [DHC, D], F32R, kind="ExternalInput")
    bq = nc.dram_tensor("bq", [DHC], F32, kind="ExternalInput")
    bk = nc.dram_tensor("bk", [DHC], F32, kind="ExternalInput")
    bv = nc.dram_tensor("bv", [DHC], F32R, kind="ExternalInput")
    maskneg = nc.dram_tensor("maskneg", [seq], F32, kind="ExternalInput")

    wT_out = nc.dram_tensor("wT", [HPC, seq, seq], F32, kind="ExternalOutput")
    recip_out = nc.dram_tensor("recip", [HPC, seq], F32, kind="ExternalOutput")
    outT = nc.dram_tensor("outT", [D, seq], F32, kind="ExternalOutput")

    with tile.TileContext(nc) as tc:
        with (
            tc.tile_pool(name="consts", bufs=1) as consts,
            tc.tile_pool(name="qkv", bufs=1) as qkv_pool,
            tc.tile_pool(name="small", bufs=1) as small,
            tc.tile_pool(name="big", bufs=big_bufs) as big_pool,
            tc.tile_pool(name="ps", bufs=2, space="PSUM") as ps,
        ):

            # persistent activation tensors
            qT_sb = qkv_pool.tile([128, 2, seq], F32R, tag="qT")
            kT_sb = qkv_pool.tile([128, 2, seq], F32R, tag="kT")
            v_sb = qkv_pool.tile([128, nkc, HPC * 65], F32R, tag="v")

            et_store = {}
            et_pool = big_pool

            deferred_dmas = []

            def emit_group(j, h, g, defer=False):
                pq = 64 * (h % 2)
                qt_h = qT_sb[pq:pq + 64, h // 2, :]
                kt_h = kT_sb[pq:pq + 64, h // 2, :]
                eg = et_pool.tile([128, GRP, qh], F32R, tag="big",
                                  name=f"eg{j}_{h}_{g}")
                for kg in range(GRP):
                    kc = g * GRP + kg
                    pscore = ps.tile([128, qh], F32, tag="ps_big",
                                     name="pscore")
                    for v5 in range(n5h):
                        nc.tensor.matmul(
                            pscore[:, v5 * 512:(v5 + 1) * 512],
                            kt_h[:, kc * 128:(kc + 1) * 128],
                            qt_h[:, j * qh + v5 * 512:j * qh + (v5 + 1) * 512],
                            start=True,
                            stop=True,
                        )
                    nc.scalar.activation(
                        eg[:, kg, :],
                        pscore[:],
                        AF.Exp,
                        bias=mask_sb[:, kc:kc + 1],
                        scale=SCALE,
                    )
                # unnormalized w^T group -> DRAM
                dst = wT_out[h, g * GRP * 128:(g + 1) * GRP * 128,
                             j * qh:(j + 1) * qh] \
                    .rearrange("(c p) q -> p c q", p=128)
                if defer:
                    deferred_dmas.append((dst, eg))
                else:
                    nc.sync.dma_start(dst, eg[:].bitcast(F32))
                et_store[(j, h, g)] = eg

            # ---- phase A: projections (Q^T, K^T first, V last) ----
            xt_pool = big_pool
            with tc.tile_pool(name="projw", bufs=1) as projw:
                wq_sb = projw.tile([128, KT, DHC], F32R, tag="wq")
                nc.sync.dma_start(wq_sb[:], wq.rearrange("(k p) n -> p k n", p=128))
                bq_sb = consts.tile([128, 2], F32, tag="bq")
                nc.sync.dma_start(bq_sb[:], bq.rearrange("(k p) -> p k", p=128))
                bk_sb = consts.tile([128, 2], F32, tag="bk")
                nc.sync.dma_start(bk_sb[:], bk.rearrange("(k p) -> p k", p=128))
                bv_sb = consts.tile([1, DHC], F32R, tag="bv")
                nc.sync.dma_start(bv_sb[:], bv[None, :])
                mask_sb = consts.tile([128, nkc], F32, tag="mask")
                nc.sync.dma_start(mask_sb[:],
                                  maskneg.rearrange("(c p) -> p c", p=128))
                onesf_sb = consts.tile([1, 128], F32, tag="onesf")
                nc.vector.memset(onesf_sb[:], 1.0)
                ones_sb = consts.tile([1, 128], F32R, tag="ones")
                nc.vector.tensor_copy(ones_sb[:], onesf_sb[:])
                onevf_sb = consts.tile([128, HPC], F32, tag="onevf")
                nc.vector.memset(onevf_sb[:], 1.0)
                onesv_sb = consts.tile([128, HPC], F32R, tag="onesv")
                nc.vector.tensor_copy(onesv_sb[:], onevf_sb[:])
                wk_sb = projw.tile([128, KT, DHC], F32R, tag="wk")
                nc.sync.dma_start(wk_sb[:], wk.rearrange("(k p) n -> p k n", p=128))
                wv_sb = projw.tile([128, KT, DHC], F32R, tag="wv")
                nc.sync.dma_start(wv_sb[:],
                                  wv.rearrange("(k p) n -> p k n", p=128))

                def proj_qk(xT_d, w_sb, b_sb, dst, name, ms):
                    xt = [xt_pool.tile([128, seq], F32R, tag="big",
                                       name=f"xt{name}{k}")
                          for k in range(KT)]
                    for k in range(KT):
                        nc.sync.dma_start(xt[k][:],
                                          xT_d[k * 128:(k + 1) * 128, :])
                    for m in ms:
                        for nn in range(nj):
                            pt = ps.tile([128, qh], F32, tag="ps_big",
                                         name="pt")
                            for v5 in range(n5h):
                                for k in range(KT):
                                    nc.tensor.matmul(
                                        pt[:, v5 * 512:(v5 + 1) * 512],
                                        w_sb[:, k, m * 128:(m + 1) * 128],
                                        xt[k][:, nn * qh + v5 * 512:
                                              nn * qh + (v5 + 1) * 512],
                                        start=(k == 0),
                                        stop=(k == KT - 1),
                                    )
                            nc.vector.tensor_scalar_add(
                                dst[:, m, nn * qh:(nn + 1) * qh],
                                pt[:],
                                b_sb[:, m:m + 1],
                            )
                        yield m

                # Q^T then K^T fully
                for _ in proj_qk(xqT, wq_sb, bq_sb, qT_sb, "q", (0, 1)):
                    pass
                for _ in proj_qk(xkT, wk_sb, bk_sb, kT_sb, "k", (0, 1)):
                    pass

                # V: [seq, HPC*65] with ones column per head; the first
                # attention block's score groups (no V dependency) are
                # emitted alongside so their w-writes overlap the xv DMA
                xt = [xt_pool.tile([128, seq], F32R, tag="big",
                                   name=f"xtv{k}")
                      for k in range(KT)]
                for k in range(KT):
                    nc.sync.dma_start(xt[k][:], xvT[k * 128:(k + 1) * 128, :])
                for g0 in range(min(early, ng)):
                    emit_group(0, 0, g0)
                for ms in range(nkc):
                    pv = ps.tile([128, DHC], F32, tag="ps_av")
                    for k in range(KT):
                        nc.tensor.matmul(
                            pv[:],
                            xt[k][:, ms * 128:(ms + 1) * 128],
                            wv_sb[:, k, :],
                            start=(k == 0),
                            stop=False,
                        )
                    nc.tensor.matmul(
                        pv[:], ones_sb[:], bv_sb[:], start=False, stop=True
                    )
                    nc.vector.tensor_copy(
                        v_sb[:, ms, :]
                        .rearrange("p (h d) -> p h d", h=HPC)[:, :, :HD],
                        pv[:].rearrange("p (h d) -> p h d", h=HPC),
                    )
                    nc.vector.tensor_copy(
                        v_sb[:, ms, :]
                        .rearrange("p (h d) -> p h d", h=HPC)[:, :, HD:],
                        onesv_sb[:, :, None],
                    )



            # ---- phase B: attention ----
            with tc.tile_pool(name="outp", bufs=2) as out_pool:
                wo_sb = out_pool.tile([128, 2, D], F32R, tag="wo", bufs=1)
                nc.sync.dma_start(wo_sb[:],
                                  wo.rearrange("(k p) n -> p k n", p=128))
                a_sb = out_pool.tile([128, 2, seq], F32R, tag="aT", bufs=1)
                for j in range(nj):
                    for h in range(HPC):
                        pq = 64 * (h % 2)
                        qt_h = qT_sb[pq:pq + 64, h // 2, :]
                        kt_h = kT_sb[pq:pq + 64, h // 2, :]
                        last = (j == nj - 1 and h == HPC - 1)
                        for g in range(ng):
                            if (j, h, g) not in et_store:
                                emit_group(j, h, g, defer=last and g >= ng - ndefer)
                        ets = [et_store.pop((j, h, g)) for g in range(ng)]

                        # AV + denominators
                        pav = ps.tile([65, qh], F32, tag="ps_av")
                        for g in range(ng):
                            for kg in range(GRP):
                                kc = g * GRP + kg
                                for v5 in range(n5h):
                                    nc.tensor.matmul(
                                        pav[:, v5 * 512:(v5 + 1) * 512],
                                        v_sb[:, kc, h * 65:(h + 1) * 65],
                                        ets[g][:, kg,
                                               v5 * 512:(v5 + 1) * 512],
                                        start=(kc == 0),
                                        stop=(kc == nkc - 1),
                                    )

                        # recip of denominators -> host + broadcast for A^T
                        recip = small.tile([1, qh], F32R, tag="recip")
                        with nc.allow_low_precision(
                            reason="f32r recip feeds f32r matmul broadcast"
                        ):
                            nc.vector.reciprocal(recip[:], pav[64:65, :])
                        nc.sync.dma_start(
                            recip_out[h, j * qh:(j + 1) * qh][None, :],
                            recip[:].bitcast(F32),
                        )
                        pb = ps.tile([64, qh], F32, tag="ps_av")
                        for v5 in range(n5h):
                            nc.tensor.matmul(
                                pb[:, v5 * 512:(v5 + 1) * 512],
                                ones_sb[:, :64],
                                recip[:, v5 * 512:(v5 + 1) * 512],
                                start=True,
                                stop=True,
                            )
                        rb = small.tile([64, qh], F32, tag="rb")
                        nc.vector.tensor_copy(rb[:], pb[:])
                        # normalize O^T into A^T (f32r, for out-projection)
                        nc.vector.tensor_tensor(
                            a_sb[pq:pq + 64, h // 2, j * qh:(j + 1) * qh],
                            pav[0:64, :],
                            rb[:],
                            MUL,
                        )

                    # pre-emit the next q-half's first score groups so the
                    # w-write stream continues across the j boundary
                    if j + 1 < nj:
                        for g0 in range(pre_j):
                            emit_group(j + 1, 0, g0)
                    # ---- output projection for this q-half ----
                    for m in range(D // 128):
                        ot = out_pool.tile([128, qh], F32, tag="ot")
                        for v5 in range(n5h):
                            po = ps.tile([128, 512], F32, tag="ps_av")
                            for k in range(2):
                                nc.tensor.matmul(
                                    po[:],
                                    wo_sb[:, k, m * 128:(m + 1) * 128],
                                    a_sb[:, k, j * qh + v5 * 512:
                                         j * qh + (v5 + 1) * 512],
                                    start=(k == 0),
                                    stop=(k == 1),
                                )
                            nc.scalar.copy(ot[:, v5 * 512:(v5 + 1) * 512],
                                           po[:])
                        nc.sync.dma_start(
                            outT[m * 128:(m + 1) * 128, j * qh:(j + 1) * qh],
                            ot[:],
                        )
                        if deferred_dmas:
                            dst, eg = deferred_dmas.pop(0)
                            nc.sync.dma_start(dst, eg[:].bitcast(F32))
                    for dst, eg in deferred_dmas:
                        nc.sync.dma_start(dst, eg[:].bitcast(F32))
                    deferred_dmas.clear()

    nc.compile()
    return nc


_NC_CACHE = {}


def _get_nc():
    if "nc" not in _NC_CACHE:
        _NC_CACHE["nc"] = build_kernel()
    return _NC_CACHE["nc"]


def shard_inputs(q, k, v, mask, Wq, bq, Wk, bk, Wv, bv, Wo):
    xT = {}
    mneg = {}
    for b in range(BS):
        xT[b] = (
            np.ascontiguousarray(q[b].T),
            np.ascontiguousarray(k[b].T),
            np.ascontiguousarray(v[b].T),
        )
        mneg[b] = np.ascontiguousarray(mask[b, 0, 0, :] * np.float32(-1e9))

    in_maps = []
    for c in range(N_CORES):
        b, t = divmod(c, TP)
        sl = slice(t * DHC, (t + 1) * DHC)
        in_maps.append(
            {
                "xqT": xT[b][0],
                "xkT": xT[b][1],
                "xvT": xT[b][2],
                "wq": np.ascontiguousarray(Wq[:, sl]),
                "wk": np.ascontiguousarray(Wk[:, sl]),
                "wv": np.ascontiguousarray(Wv[:, sl]),
                "wo": np.ascontiguousarray(Wo[sl, :]),
                "bq": np.ascontiguousarray(bq[sl]),
                "bk": np.ascontiguousarray(bk[sl]),
                "bv": np.ascontiguousarray(bv[sl]),
                "maskneg": mneg[b],
            }
        )
    return in_maps


def unshard_outputs(results, bo):
    out = np.empty((BS, SEQ, D), dtype=np.float32)
    w = np.empty((BS, H, SEQ, SEQ), dtype=np.float32)
    acc = np.zeros((BS, D, SEQ), dtype=np.float32)
    for c in range(N_CORES):
        b, t = divmod(c, TP)
        r = results[c]
        acc[b] += r["outT"]
        wt = r["wT"]  # [HPC, k, q] unnormalized
        rc = r["recip"]  # [HPC, q]
        for h in range(HPC):
            # w[q, k] = E^T[k, q].T * recip[q, None]
            np.multiply(wt[h].T, rc[h][:, None], out=w[b, t * HPC + h])
    for b in range(BS):
        out[b] = acc[b].T + bo[None, :]
    return out, w


def kernel(q, k, v, mask, Wq, bq, Wk, bk, Wv, bv, Wo, bo):
    args = [np.asarray(a, dtype=np.float32)
            for a in (q, k, v, mask, Wq, bq, Wk, bk, Wv, bv, Wo, bo)]
    q, k, v, mask, Wq, bq, Wk, bk, Wv, bv, Wo, bo = args

    nc = _get_nc()
    in_maps = shard_inputs(q, k, v, mask, Wq, bq, Wk, bk, Wv, bv, Wo)
    res = run_bass_kernel_spmd(nc, in_maps, list(range(N_CORES)))
    return unshard_outputs(res.results, bo)
